# revision 80
# baseline (speedup 1.0000x reference)
"""Trainium2 Bass kernel for nn_MultiHeadMLPAttentionModel — K=128-dense design.

All matmuls use full-K=128 operands (zero-padded stationaries / zero-padded
weight rhs) so the PE's HAM activity monitor sees dense streams and holds the
clock at 2.4 GHz for the whole kernel (K=8 row-tiled matmuls read as "idle"
and get throttled to 1.2 GHz — measured). fp8 DoubleRow was tried and is a
LOSS here: on HW it streams at the same 1 col/cycle but serializes its
doubled LDWEIGHTS (~53ns/MM), and our real contraction is only K=8.

Per core: 16 batch rows as 8 b-pairs. Point data is replicated across all
128 SBUF partitions (8 copies of each b's 8 feature rows); stationaries are
zero except the 8 rows matching their b's home slot, so K=128 contraction
reproduces the K=8 result at identical stream cost.

Per chunk (512 points, both b's of a pair), issued sh -> logits -> pool ->
h1 on the PE:
  * score hiddens: 4 K=128 fp8 matmuls into sAB [128,1024] (2 banks,
    relu'd by scalar as one op) + sC/sD [128,512] (from a 3-buf rotation so
    relu lateness never gates the next chunk; relu'd by vector). Relu ->
    fp8 sh_sb, buffered a full pair ahead of the logits.
  * logits point-major: 16 tiny N=4 matmuls, lhsT = sh slices, rhs =
    zero-padded w2 columns; accumulate into a quarter-pair [128,128] PSUM
    tile (1 bank) -> exp every 4 chunks -> bf16 enm.
  * pooling of the PREVIOUS pair (8 matmuls): 65-wide h1a strips whose
    count column is a one-time SBUF memset (1.0) — normalizers MUST ride
    the same accumulating matmuls because any other group's start flag
    zeroes its whole 2KB PSUM bank (lazy zero region = 2048B).
  * encoder h1 point-major: 8 matmuls into one [128,512] 1-bank PSUM tile,
    one relu per chunk alternating vector/scalar -> bf16 strips.
Engine balance per chunk: T ~1.75us, V (sC+sD relus + h1 every other
chunk) ~1.7us, S (sAB relu + exp + epilogue) ~1.5us with slack to absorb
pair-boundary bursts.
Startup: one fused critical DMA (pair-0 stationaries + w24) + quarter of
xa0 (issued from the scalar queue) lets chunk (0,0) start ~11.4us; all
other DMAs are pushed back via tile_wait_until so they don't steal HBM
bandwidth.
Phase D (once): enc layer 2 + output MLP on pooled contexts -> [16].
"""

import numpy as np

import concourse.bass as bass
import concourse.tile as tile
from concourse import bacc, mybir

B, N, HID, HEADS = 128, 8192, 64, 4
NCORES = 8
BPC = B // NCORES      # 16 batch rows per core
PAIRS = BPC // 2       # 8 b-pairs
NCHB = N // 512        # 16 chunks per pair
NB = N // 128          # 64 point-blocks per b
NSTR = 16              # h1 strips per b (4 blocks each)

F32 = mybir.dt.float32
BF16 = mybir.dt.bfloat16
FP8 = mybir.dt.float8e4
AF = mybir.ActivationFunctionType
ALU = mybir.AluOpType


def build_nc():
    from contextlib import ExitStack

    nc = bacc.Bacc()
    f32 = F32

    xpa_d = nc.dram_tensor("xpa", [PAIRS, 128, N], FP8, kind="ExternalInput")
    crit_d = nc.dram_tensor("crit", [128, 648], FP8, kind="ExternalInput")
    wsc_d = nc.dram_tensor("wsc", [128, PAIRS * 512], FP8, kind="ExternalInput")
    wenm_d = nc.dram_tensor("wenm", [128, PAIRS * 128], FP8, kind="ExternalInput")
    ew2b_d = nc.dram_tensor("ew2b", [65, 64], f32, kind="ExternalInput")
    ow1_d = nc.dram_tensor("ow1", [64, 256], f32, kind="ExternalInput")
    ob1_d = nc.dram_tensor("ob1", [1, 64], f32, kind="ExternalInput")
    w2o_d = nc.dram_tensor("w2o", [65, 1], f32, kind="ExternalInput")
    id4_d = nc.dram_tensor("id4", [4, 4], f32, kind="ExternalInput")
    on16_d = nc.dram_tensor("on16", [1, BPC], f32, kind="ExternalInput")
    out_d = nc.dram_tensor("out", [BPC], f32, kind="ExternalOutput")

    with tile.TileContext(nc) as tc, ExitStack() as ctx:
        consts = ctx.enter_context(tc.tile_pool(name="consts", bufs=1))

        def ctile(shape, nm, dt=f32):
            return consts.tile(shape, dt, name=nm, tag=nm)

        crit = ctile([128, 648], "crit", FP8)
        wsc = ctile([128, PAIRS * 512], "wsc", FP8)
        wenm = ctile([128, PAIRS * 128], "wenm", FP8)
        ew2b = ctile([65, 64], "ew2b")
        ow1 = ctile([64, 256], "ow1")
        ob1 = ctile([1, 64], "ob1")
        w2o = ctile([65, 1], "w2o")
        id4 = ctile([4, 4], "id4")
        on16 = ctile([1, BPC], "on16")

        ctxnT = consts.tile([65, 64], f32, name="ctxnT", tag="ctxnT")
        obuf = consts.tile([65, BPC], f32, name="obuf", tag="obuf")
        fct = consts.tile([64, 64], f32, name="fct", tag="fct")
        res = consts.tile([1, BPC], f32, name="res", tag="res")

        # Single fused pair-0-critical DMA (pair-0 stationaries + w24) so
        # chunk (0,0) can start ~immediately; everything else is issued
        # after xa0/xa1 below.
        nc.sync.dma_start(crit[:], crit_d[:])
        nc.vector.memset(ctxnT[64:65, :], 1.0)
        nc.vector.memset(obuf[64:65, :], 1.0)

        with ExitStack() as pctx:
            xapool = pctx.enter_context(tc.tile_pool(name="xap", bufs=3))
            shpool = pctx.enter_context(tc.tile_pool(name="shp", bufs=18))
            h1pool = pctx.enter_context(tc.tile_pool(name="h1p", bufs=3))
            enmpool = pctx.enter_context(tc.tile_pool(name="enmp", bufs=3))
            smpool = pctx.enter_context(tc.tile_pool(name="smp", bufs=4))
            # PSUM map (8 banks): sAB [128,1024] (2 banks, bufs=1) + sC/sD
            # [128,512] (bufs=3 -> 3 banks, 1.5-chunk reuse distance) +
            # quarter-pair logits (1) + merged h1 (1) + cx (1).
            psS2 = pctx.enter_context(tc.tile_pool(name="psS2", bufs=1, space="PSUM"))
            psS1 = pctx.enter_context(tc.tile_pool(name="psS1", bufs=3, space="PSUM"))
            psLG = pctx.enter_context(tc.tile_pool(name="psLG", bufs=1, space="PSUM"))
            psH = pctx.enter_context(tc.tile_pool(name="psH", bufs=1, space="PSUM"))
            psCX = pctx.enter_context(tc.tile_pool(name="psCX", bufs=1, space="PSUM"))

            xas = {}

            def load_xa(p):
                t = xapool.tile([128, N], FP8, name="xa", tag="xa")
                nc.sync.dma_start(t[:], xpa_d[p])
                xas[p] = t

            # xa0: allocate now, but only DMA the first quarter — chunks 0-3
            # need just that, and the tail quarters are issued after the DMA
            # throttle below.
            xa0 = xapool.tile([128, N], FP8, name="xa", tag="xa")
            nc.scalar.dma_start(xa0[:, 0 : N // 4], xpa_d[0][:, 0 : N // 4])
            xas[0] = xa0

            def late_dmas():
                nc.sync.dma_start(wsc[:, 512:], wsc_d[:, 512:])
                nc.sync.dma_start(wenm[:, 128:], wenm_d[:, 128:])
                nc.sync.dma_start(ew2b[:], ew2b_d[:])
                nc.sync.dma_start(ow1[:], ow1_d[:])
                nc.sync.dma_start(ob1[:], ob1_d[:])
                nc.sync.dma_start(w2o[:], w2o_d[:])
                nc.sync.dma_start(id4[:], id4_d[:])
                nc.sync.dma_start(on16[:], on16_d[:])

            def pool_mms(cx, enm, h1a, ts):
                # enm cols are quarter-major: [Q(4)][i(2)][t_local(16)][h(4)].
                # h1a strips are 65 wide (64 hidden + memset count col), so
                # the normalizers accumulate inside these same matmuls — a
                # separate count accumulation would be wiped by other groups'
                # start flags (PSUM start zeroes its whole 2KB bank).
                for t in ts:
                    for i in (0, 1):
                        ec = 128 * (t // 16) + 64 * i + 4 * (t % 16)
                        k = 8 * (t // 4) + 2 * (t % 4) + i
                        nc.tensor.matmul(
                            cx[0:4, 65 * i : 65 * i + 65],
                            enm[:, ec : ec + 4],
                            h1a[:, 65 * k : 65 * k + 65],
                            start=(t == 0), stop=(t == NB - 1),
                            skip_group_check=True,
                        )

            def epilogue(cx, p):
                rz = smpool.tile([4, 2], f32, name="rz", tag="rz")
                nc.vector.reciprocal(rz[:, 0:1], cx[0:4, 64:65])
                nc.vector.reciprocal(rz[:, 1:2], cx[0:4, 129:130])
                cpn = smpool.tile([4, 128], f32, name="cpn", tag="cpn")
                nc.scalar.activation(cpn[:, 0:64], cx[0:4, 0:64], AF.Copy,
                                     scale=rz[:, 0:1])
                nc.scalar.activation(cpn[:, 64:128], cx[0:4, 65:129], AF.Copy,
                                     scale=rz[:, 1:2])
                tp = cx[:, 130:134]
                nc.tensor.transpose(tp[:], cpn[:], id4[:])
                nc.scalar.activation(
                    ctxnT[0:64, 8 * p : 8 * p + 4], tp[0:64, :], AF.Copy
                )
                nc.scalar.activation(
                    ctxnT[0:64, 8 * p + 4 : 8 * p + 8], tp[64:128, :], AF.Copy
                )

            # Software-pipelined across pairs: in super-iteration q,
            # pair q+1 runs its score matmuls + relus (sh_sb buffered in
            # SBUF for a full pair), pair q runs logits + h1, pair q-1 runs
            # pooling. Removes the relu -> logit latency from the chunk
            # critical path entirely.
            shs = {}   # (pair, chunk) -> relu'd sh_sb tile
            lgs = {}
            enms = {}
            h1as = {}
            cxs = {}

            def sh_step(p, cc):
                # Two 2-bank PSUM tiles per chunk; each relu is issued as
                # soon as its two matmuls land (V takes sAB, S takes sCD), so
                # the relu -> next-chunk-MM cycle (~432+1224+sem) just fits
                # inside the tensor chunk period.
                cs = slice(512 * cc, 512 * (cc + 1))
                xa = xas[p]
                sh_sb = shpool.tile([128, 2048], FP8, name="sh_sb", tag="shs")
                wsp = crit if p == 0 else wsc
                wso = 0 if p == 0 else 512 * p
                sAB = psS2.tile([128, 1024], f32, name="sAB", tag="sAB")
                for s in (0, 1):
                    nc.tensor.matmul(
                        sAB[:, 512 * s : 512 * s + 512],
                        wsp[:, wso + 128 * s : wso + 128 * s + 128],
                        xa[:, cs],
                        start=True, stop=True,
                        skip_group_check=True,
                    )
                nc.scalar.activation(sh_sb[:, 0:1024], sAB[:], AF.Relu)
                for s in (2, 3):
                    sq = psS1.tile([128, 512], f32, name="sq", tag="sq")
                    nc.tensor.matmul(
                        sq[:],
                        wsp[:, wso + 128 * s : wso + 128 * s + 128],
                        xa[:, cs],
                        start=True, stop=True,
                        skip_group_check=True,
                    )
                    nc.vector.tensor_scalar(
                        sh_sb[:, 512 * s : 512 * s + 512], sq[:], 0.0, None,
                        ALU.max,
                    )
                shs[(p, cc)] = sh_sb

            def lg_step(p, cc):
                # Quarter-pair logits: a [128,128] PSUM tile holds 4 chunks'
                # logits, exp'd into its enm slice as soon as it completes.
                sh_sb = shs.pop((p, cc))
                if cc % 4 == 0:
                    lgs[p] = psLG.tile([128, 128], f32, name="lgq", tag="lgq")
                lg = lgs[p]
                for i in (0, 1):
                    for j in range(4):
                        col = 64 * i + 16 * (cc % 4) + 4 * j
                        lgo = lg[:, col : col + 4]
                        base = 1024 * i
                        nc.tensor.matmul(
                            lgo,
                            sh_sb[:, base + 128 * j : base + 128 * j + 128],
                            crit[:, 640:644],
                            start=True, stop=False,
                            skip_group_check=True,
                        )
                        nc.tensor.matmul(
                            lgo,
                            sh_sb[:, base + 512 + 128 * j : base + 512 + 128 * j + 128],
                            crit[:, 644:648],
                            start=False, stop=True,
                            skip_group_check=True,
                        )


            def h1_step(p, cc):
                xa = xas[p]
                h1a = h1as[p]
                h1ab = psH.tile([128, 512], f32, name="h1ab", tag="h1ab")
                wnp = crit if p == 0 else wenm
                wno = 512 if p == 0 else 128 * p
                for j in range(4):
                    t = 4 * cc + j
                    nc.tensor.matmul(
                        h1ab[:, 128 * j : 128 * j + 128],
                        xa[:, 128 * t : 128 * t + 128],
                        wnp[:, wno : wno + 128],
                        start=True, stop=True,
                        skip_group_check=True,
                    )
                dst = h1a.rearrange("p (k c) -> p k c", c=65)[
                    :, 8 * cc : 8 * cc + 8, 0:64
                ]
                # V takes h1 only on flush-exp chunks (1 in 4): a V h1-relu
                # spikes V to ~2.04us (> T period) and with 1-in-2 spacing
                # the backlog never drained (repeating ~2.7us chunks); 3
                # light chunks between spikes let V recover, and S skips h1
                # exactly where its exp lands.
                if cc % 4 == 0:
                    nc.vector.tensor_scalar(dst, h1ab[:], 0.0, None, ALU.max)
                else:
                    nc.scalar.activation(dst, h1ab[:], AF.Relu)

            def open_pair(p):
                enms[p] = enmpool.tile([128, 512], BF16, name="enm", tag="enm")
                h1as[p] = h1pool.tile([128, 128 * 65], BF16, name="h1a", tag="h1a")
                if p < 3:
                    # one-time 1.0 fill of the count columns of each of the
                    # 3 rotating physical buffers; relus never touch them
                    nc.vector.memset(
                        h1as[p].rearrange("p (k c) -> p k c", c=65)[:, :, 64:65],
                        1.0,
                    )

            def flush_exp(p, qq):
                # exp of a completed logit quarter; issued at the TOP of the
                # next chunk so the scalar engine runs it before its relu
                # backlog (frees the single lgq bank for the next quarter)
                nc.scalar.activation(
                    enms[p][:, 128 * qq : 128 * qq + 128], lgs.pop(p)[:],
                    AF.Exp,
                )

            LAG = 8   # pool trails its logits by 8 chunks: each enm
            # quarter's exp gets a 4-chunk margin before pooling reads it
            # (at LAG=6 the quarter-transition pool MMs stalled ~900ns on S)
            open_pair(0)
            sh_step(0, 0)
            # The tile scheduler orders by readiness, so to keep these
            # non-critical DMAs from stealing HBM bandwidth during startup,
            # push their scheduler-visible ready time out a few microseconds.
            with tc.tile_wait_until(0.003):
                for k in (1, 2, 3):
                    sl = slice(k * (N // 4), (k + 1) * (N // 4))
                    nc.sync.dma_start(xas[0][:, sl], xpa_d[0][:, sl])
            with tc.tile_wait_until(0.006):
                if PAIRS > 1:
                    load_xa(1)
                late_dmas()
            for cc in range(1, NCHB):
                sh_step(0, cc)
            for q in range(PAIRS):
                # super-iteration q: sh for q+1, logits/h1 for q, pool for q
                # (LAG chunks behind), tail of pool(q-1) + its epilogue in
                # the first LAG chunks.
                if q + 1 < PAIRS:
                    open_pair(q + 1)
                if q + 2 < PAIRS:
                    load_xa(q + 2)
                for cc in range(NCHB):
                    if cc % 4 == 0 and cc > 0:
                        flush_exp(q, cc // 4 - 1)
                    if q + 1 < PAIRS:
                        sh_step(q + 1, cc)
                    lg_step(q, cc)
                    if q >= 1 and cc < LAG:
                        c2 = NCHB - LAG + cc
                        pool_mms(cxs[q - 1], enms[q - 1], h1as[q - 1],
                                 (4 * c2, 4 * c2 + 1, 4 * c2 + 2, 4 * c2 + 3))
                        if cc == LAG - 1:
                            epilogue(cxs.pop(q - 1), q - 1)
                            enms.pop(q - 1)
                            h1as.pop(q - 1)
                    if cc >= LAG:
                        if cc == LAG:
                            cxs[q] = psCX.tile([128, 140], f32, name="cx",
                                               tag="cx")
                        c2 = cc - LAG
                        pool_mms(cxs[q], enms[q], h1as[q],
                                 (4 * c2, 4 * c2 + 1, 4 * c2 + 2, 4 * c2 + 3))
                    h1_step(q, cc)
                flush_exp(q, 3)
                if q - 1 >= 0:
                    xas.pop(q - 1)

            qf = PAIRS - 1
            for cc in range(NCHB - LAG, NCHB):
                pool_mms(cxs[qf], enms[qf], h1as[qf],
                         (4 * cc, 4 * cc + 1, 4 * cc + 2, 4 * cc + 3))
            epilogue(cxs.pop(qf), qf)

        # ---- Phase D: pooled-context encoder layer 2 + output MLP
        with ExitStack() as pctx:
            psD = pctx.enter_context(tc.tile_pool(name="psD", bufs=1, space="PSUM"))
            fct_ps = psD.tile([64, 64], f32, name="fct_ps", tag="fctp")
            nc.tensor.matmul(fct_ps[:], ew2b[:], ctxnT[:], start=True, stop=True)
            nc.vector.tensor_copy(out=fct[:], in_=fct_ps[:])
            fct_bh = fct.rearrange("d (b h) -> d b h", h=HEADS)
            o1_ps = psD.tile([64, BPC], f32, name="o1_ps", tag="o1p")
            for h in range(HEADS):
                nc.tensor.matmul(
                    o1_ps[:],
                    ow1[:, h * 64 : (h + 1) * 64],
                    fct_bh[:, :, h],
                    start=(h == 0),
                    stop=False,
                    skip_group_check=True,
                )
            nc.tensor.matmul(
                o1_ps[:], ob1[:], on16[:], start=False, stop=True,
                skip_group_check=True,
            )
            nc.scalar.activation(obuf[0:64, :], o1_ps[:], AF.Relu)
            fin_ps = psD.tile([1, BPC], f32, name="fin_ps", tag="finp")
            nc.tensor.matmul(fin_ps[:], w2o[:], obuf[:], start=True, stop=True)
            nc.vector.tensor_copy(out=res[:], in_=fin_ps[:])
            nc.sync.dma_start(out_d.rearrange("(a n) -> a n", a=1), res[:])

    if not nc.is_finalized():
        nc.finalize()
    return nc


def make_in_maps(inputs):
    """Host-side marshalling: fp8 hi/lo packing; radar folded into biases;
    point rows replicated across all 128 partitions (8 copies per b)."""
    import ml_dtypes

    f8 = ml_dtypes.float8_e4m3fn
    f = np.float32

    def split8(a):
        hi = a.astype(f8)
        lo = (a - hi.astype(f)).astype(f8)
        return hi, lo

    radar = np.concatenate(
        [np.asarray(inputs["radar_xy"], f), np.asarray(inputs["radar_dir"], f)], axis=1
    )
    pts = np.asarray(inputs["pts"], f)
    enc_w1 = np.asarray(inputs["enc_w1"], f)
    enc_b1 = np.asarray(inputs["enc_b1"], f)
    enc_w2 = np.asarray(inputs["enc_w2"], f)
    enc_b2 = np.asarray(inputs["enc_b2"], f)
    sc_w1 = np.asarray(inputs["sc_w1"], f)
    sc_b1 = np.asarray(inputs["sc_b1"], f)
    sc_w2 = np.asarray(inputs["sc_w2"], f)
    out_w1 = np.asarray(inputs["out_w1"], f)
    out_b1 = np.asarray(inputs["out_b1"], f)
    out_w2 = np.asarray(inputs["out_w2"], f)
    out_b2 = np.asarray(inputs["out_b2"], f)

    cb_sc = np.einsum("br,hrd->bhd", radar, sc_w1[:, :4, :]) + sc_b1  # [B, 4, 64]
    cb_enc = radar @ enc_w1[:4] + enc_b1  # [B, 64]

    x = pts[:, :, 0]
    y = pts[:, :, 1]
    xh, xl = split8(x)
    yh, yl = split8(y)

    # 8 feature rows per b: [xh, yh, xh, yh, xl, yl, 1, 1]
    xrows = np.empty((B, 8, N), f8)
    xrows[:, 0] = xh
    xrows[:, 1] = yh
    xrows[:, 2] = xh
    xrows[:, 3] = yh
    xrows[:, 4] = xl
    xrows[:, 5] = yl
    xrows[:, 6] = 1.0
    xrows[:, 7] = 1.0

    # score stationary rows: [wxh, wyh, wxl, wyl, wxh, wyh, cbh, cbl]
    wx = sc_w1[:, 4, :]
    wy = sc_w1[:, 5, :]
    wxh_, wxl_ = split8(wx)
    wyh_, wyl_ = split8(wy)

    def sc_stat(b, half, row0):
        st = np.zeros((128, 128), f8)
        for hh in range(2):
            h = half * 2 + hh
            s = slice(hh * 64, hh * 64 + 64)
            st[row0 + 0, s] = wxh_[h]
            st[row0 + 1, s] = wyh_[h]
            st[row0 + 2, s] = wxl_[h]
            st[row0 + 3, s] = wyl_[h]
            st[row0 + 4, s] = wxh_[h]
            st[row0 + 5, s] = wyh_[h]
            cbh, cbl = split8(cb_sc[b, h])
            st[row0 + 6, s] = cbh
            st[row0 + 7, s] = cbl
        return st

    exh_, exl_ = split8(enc_w1[4])
    eyh_, eyl_ = split8(enc_w1[5])

    def enc_rhs(b, row0):
        st = np.zeros((128, 64), f8)
        st[row0 + 0] = exh_
        st[row0 + 1] = eyh_
        st[row0 + 2] = exl_
        st[row0 + 3] = eyl_
        st[row0 + 4] = exh_
        st[row0 + 5] = eyh_
        cbh, cbl = split8(cb_enc[b])
        st[row0 + 6] = cbh
        st[row0 + 7] = cbl
        return st

    w24 = np.zeros((128, 8), f8)
    w24[0:64, 0] = sc_w2[0].astype(f8)
    w24[64:128, 1] = sc_w2[1].astype(f8)
    w24[0:64, 6] = sc_w2[2].astype(f8)
    w24[64:128, 7] = sc_w2[3].astype(f8)

    ew2b = np.concatenate([enc_w2, enc_b2[None, :]], axis=0)
    ow1 = np.empty((64, 256), f)
    for h in range(HEADS):
        ow1[:, h * 64 : (h + 1) * 64] = out_w1[h * 64 : (h + 1) * 64, :]
    ob1 = np.ascontiguousarray(out_b1[None, :])
    w2o = np.concatenate([out_w2, out_b2[None, :]], axis=0)
    id4 = np.eye(4, dtype=f)
    on16 = np.ones((1, BPC), f)

    in_maps = []
    for core in range(NCORES):
        b0 = core * BPC
        xpa = np.empty((PAIRS, 128, N), f8)
        wsc = np.empty((128, PAIRS * 512), f8)
        wenm = np.empty((128, PAIRS * 128), f8)
        for p in range(PAIRS):
            be, bo = b0 + 2 * p, b0 + 2 * p + 1
            xpa[p, 0:64] = np.tile(xrows[be], (8, 1))
            xpa[p, 64:128] = np.tile(xrows[bo], (8, 1))
            wsc[:, 512 * p + 0 : 512 * p + 128] = sc_stat(be, 0, 0)
            wsc[:, 512 * p + 128 : 512 * p + 256] = sc_stat(be, 1, 0)
            wsc[:, 512 * p + 256 : 512 * p + 384] = sc_stat(bo, 0, 64)
            wsc[:, 512 * p + 384 : 512 * p + 512] = sc_stat(bo, 1, 64)
            wenm[:, 128 * p : 128 * p + 64] = enc_rhs(be, 0)
            wenm[:, 128 * p + 64 : 128 * p + 128] = enc_rhs(bo, 64)
        critb = np.concatenate([wsc[:, 0:512], wenm[:, 0:128],
                                w24.reshape(128, 8)], axis=1)
        in_maps.append(
            dict(
                xpa=xpa, crit=critb, wsc=wsc, wenm=wenm, ew2b=ew2b, ow1=ow1,
                ob1=ob1, w2o=w2o, id4=id4, on16=on16,
            )
        )
    return in_maps


_CACHE = {}


def _get_runner():
    if "runner" in _CACHE:
        return _CACHE["runner"]

    import jax
    from jax.sharding import Mesh, NamedSharding, PartitionSpec

    from concourse.bass2jax import (
        _bass_exec_p,
        install_neuronx_cc_hook,
        partition_id_tensor,
        shard_map,
    )

    nc = build_nc()
    _CACHE["nc"] = nc
    install_neuronx_cc_hook()
    partition_name = nc.partition_id_tensor.name if nc.partition_id_tensor else None
    in_names, out_names, out_avals = [], [], []
    for alloc in nc.m.functions[0].allocations:
        if not isinstance(alloc, mybir.MemoryLocationSet):
            continue
        name = alloc.memorylocations[0].name
        if alloc.kind == "ExternalInput":
            if name != partition_name:
                in_names.append(name)
        elif alloc.kind == "ExternalOutput":
            out_names.append(name)
            out_avals.append(
                jax.core.ShapedArray(tuple(alloc.tensor_shape), mybir.dt.np(alloc.dtype))
            )
    all_in_names = tuple(in_names + out_names)
    if partition_name is not None:
        all_in_names = all_in_names + (partition_name,)

    def _body(*args):
        operands = list(args)
        if partition_name is not None:
            operands.append(partition_id_tensor())
        return tuple(
            _bass_exec_p.bind(
                *operands,
                out_avals=tuple(out_avals),
                in_names=all_in_names,
                out_names=tuple(out_names),
                lowering_input_output_aliases=(),
                sim_require_finite=True,
                sim_require_nnan=True,
                nc=nc,
            )
        )

    devices = jax.devices()[:NCORES]
    mesh = Mesh(np.asarray(devices), ("core",))
    nin = len(in_names) + len(out_names)
    fn = jax.jit(
        shard_map(
            _body,
            mesh=mesh,
            in_specs=(PartitionSpec("core"),) * nin,
            out_specs=(PartitionSpec("core"),) * len(out_names),
            check_rep=False,
        ),
        keep_unused=True,
    )
    sharding = NamedSharding(mesh, PartitionSpec("core"))
    runner = (fn, sharding, in_names, out_avals)
    _CACHE["runner"] = runner
    return runner


def kernel(**inputs):
    import jax

    in_maps = make_in_maps(inputs)
    fn, sharding, in_names, out_avals = _get_runner()
    concat_in = [
        np.concatenate([np.asarray(in_maps[c][name]) for c in range(NCORES)], axis=0)
        for name in in_names
    ]
    concat_zeros = [
        np.zeros((NCORES * a.shape[0], *a.shape[1:]), a.dtype) for a in out_avals
    ]
    args = [jax.device_put(a, sharding) for a in (*concat_in, *concat_zeros)]
    (out,) = fn(*args)
    return np.asarray(out).reshape(B).astype(np.float32)



# revision 81
# speedup vs baseline: 1.0019x; 1.0019x over previous
"""Trainium2 Bass kernel for nn_MultiHeadMLPAttentionModel — K=128-dense design.

All matmuls use full-K=128 operands (zero-padded stationaries / zero-padded
weight rhs) so the PE's HAM activity monitor sees dense streams and holds the
clock at 2.4 GHz for the whole kernel (K=8 row-tiled matmuls read as "idle"
and get throttled to 1.2 GHz — measured). fp8 DoubleRow was tried and is a
LOSS here: on HW it streams at the same 1 col/cycle but serializes its
doubled LDWEIGHTS (~53ns/MM), and our real contraction is only K=8.

Per core: 16 batch rows as 8 b-pairs. Point data is replicated across all
128 SBUF partitions (8 copies of each b's 8 feature rows); stationaries are
zero except the 8 rows matching their b's home slot, so K=128 contraction
reproduces the K=8 result at identical stream cost.

Per chunk (512 points, both b's of a pair), issued sh -> logits -> pool ->
h1 on the PE:
  * score hiddens: 4 K=128 fp8 matmuls into sAB [128,1024] (2 banks,
    relu'd by scalar as one op) + sC/sD [128,512] (from a 3-buf rotation so
    relu lateness never gates the next chunk; relu'd by vector). Relu ->
    fp8 sh_sb, buffered a full pair ahead of the logits.
  * logits point-major: 16 tiny N=4 matmuls, lhsT = sh slices, rhs =
    zero-padded w2 columns; accumulate into a quarter-pair [128,128] PSUM
    tile (1 bank) -> exp every 4 chunks -> bf16 enm.
  * pooling of the PREVIOUS pair (8 matmuls): 65-wide h1a strips whose
    count column is a one-time SBUF memset (1.0) — normalizers MUST ride
    the same accumulating matmuls because any other group's start flag
    zeroes its whole 2KB PSUM bank (lazy zero region = 2048B).
  * encoder h1 point-major: 8 matmuls into one [128,512] 1-bank PSUM tile,
    one relu per chunk alternating vector/scalar -> bf16 strips.
Engine balance per chunk: T ~1.75us, V (sC+sD relus + h1 every other
chunk) ~1.7us, S (sAB relu + exp + epilogue) ~1.5us with slack to absorb
pair-boundary bursts.
Startup: one fused critical DMA (pair-0 stationaries + w24) + quarter of
xa0 (issued from the scalar queue) lets chunk (0,0) start ~11.4us; all
other DMAs are pushed back via tile_wait_until so they don't steal HBM
bandwidth.
Phase D (once): enc layer 2 + output MLP on pooled contexts -> [16].
"""

import numpy as np

import concourse.bass as bass
import concourse.tile as tile
from concourse import bacc, mybir

B, N, HID, HEADS = 128, 8192, 64, 4
NCORES = 8
BPC = B // NCORES      # 16 batch rows per core
PAIRS = BPC // 2       # 8 b-pairs
NCHB = N // 512        # 16 chunks per pair
NB = N // 128          # 64 point-blocks per b
NSTR = 16              # h1 strips per b (4 blocks each)

F32 = mybir.dt.float32
BF16 = mybir.dt.bfloat16
FP8 = mybir.dt.float8e4
AF = mybir.ActivationFunctionType
ALU = mybir.AluOpType


def build_nc():
    from contextlib import ExitStack

    nc = bacc.Bacc()
    f32 = F32

    xpa_d = nc.dram_tensor("xpa", [PAIRS, 128, N], FP8, kind="ExternalInput")
    crit_d = nc.dram_tensor("crit", [128, 648], FP8, kind="ExternalInput")
    wsc_d = nc.dram_tensor("wsc", [128, PAIRS * 512], FP8, kind="ExternalInput")
    wenm_d = nc.dram_tensor("wenm", [128, PAIRS * 128], FP8, kind="ExternalInput")
    ew2b_d = nc.dram_tensor("ew2b", [65, 64], f32, kind="ExternalInput")
    ow1_d = nc.dram_tensor("ow1", [64, 256], f32, kind="ExternalInput")
    ob1_d = nc.dram_tensor("ob1", [1, 64], f32, kind="ExternalInput")
    w2o_d = nc.dram_tensor("w2o", [65, 1], f32, kind="ExternalInput")
    id4_d = nc.dram_tensor("id4", [4, 4], f32, kind="ExternalInput")
    on16_d = nc.dram_tensor("on16", [1, BPC], f32, kind="ExternalInput")
    out_d = nc.dram_tensor("out", [BPC], f32, kind="ExternalOutput")

    with tile.TileContext(nc) as tc, ExitStack() as ctx:
        consts = ctx.enter_context(tc.tile_pool(name="consts", bufs=1))

        def ctile(shape, nm, dt=f32):
            return consts.tile(shape, dt, name=nm, tag=nm)

        crit = ctile([128, 648], "crit", FP8)
        wsc = ctile([128, PAIRS * 512], "wsc", FP8)
        wenm = ctile([128, PAIRS * 128], "wenm", FP8)
        ew2b = ctile([65, 64], "ew2b")
        ow1 = ctile([64, 256], "ow1")
        ob1 = ctile([1, 64], "ob1")
        w2o = ctile([65, 1], "w2o")
        id4 = ctile([4, 4], "id4")
        on16 = ctile([1, BPC], "on16")

        ctxnT = consts.tile([65, 64], f32, name="ctxnT", tag="ctxnT")
        obuf = consts.tile([65, BPC], f32, name="obuf", tag="obuf")
        fct = consts.tile([64, 64], f32, name="fct", tag="fct")
        res = consts.tile([1, BPC], f32, name="res", tag="res")

        # Single fused pair-0-critical DMA (pair-0 stationaries + w24) so
        # chunk (0,0) can start ~immediately; everything else is issued
        # after xa0/xa1 below.
        nc.sync.dma_start(crit[:], crit_d[:])
        nc.vector.memset(ctxnT[64:65, :], 1.0)
        nc.vector.memset(obuf[64:65, :], 1.0)

        with ExitStack() as pctx:
            xapool = pctx.enter_context(tc.tile_pool(name="xap", bufs=3))
            shpool = pctx.enter_context(tc.tile_pool(name="shp", bufs=18))
            h1pool = pctx.enter_context(tc.tile_pool(name="h1p", bufs=3))
            enmpool = pctx.enter_context(tc.tile_pool(name="enmp", bufs=3))
            smpool = pctx.enter_context(tc.tile_pool(name="smp", bufs=4))
            # PSUM map (8 banks): sAB [128,1024] (2 banks, bufs=1) + sC/sD
            # [128,512] (bufs=3 -> 3 banks, 1.5-chunk reuse distance) +
            # quarter-pair logits (1) + merged h1 (1) + cx (1).
            psS2 = pctx.enter_context(tc.tile_pool(name="psS2", bufs=1, space="PSUM"))
            psS1 = pctx.enter_context(tc.tile_pool(name="psS1", bufs=3, space="PSUM"))
            psLG = pctx.enter_context(tc.tile_pool(name="psLG", bufs=1, space="PSUM"))
            psH = pctx.enter_context(tc.tile_pool(name="psH", bufs=1, space="PSUM"))
            psCX = pctx.enter_context(tc.tile_pool(name="psCX", bufs=1, space="PSUM"))

            xas = {}

            def load_xa(p):
                t = xapool.tile([128, N], FP8, name="xa", tag="xa")
                nc.sync.dma_start(t[:], xpa_d[p])
                xas[p] = t

            # xa0: allocate now, but only DMA the first quarter — chunks 0-3
            # need just that, and the tail quarters are issued after the DMA
            # throttle below.
            xa0 = xapool.tile([128, N], FP8, name="xa", tag="xa")
            nc.scalar.dma_start(xa0[:, 0 : N // 4], xpa_d[0][:, 0 : N // 4])
            xas[0] = xa0

            def late_dmas():
                nc.sync.dma_start(wsc[:, 512:], wsc_d[:, 512:])
                nc.sync.dma_start(wenm[:, 128:], wenm_d[:, 128:])
                nc.sync.dma_start(ew2b[:], ew2b_d[:])
                nc.sync.dma_start(ow1[:], ow1_d[:])
                nc.sync.dma_start(ob1[:], ob1_d[:])
                nc.sync.dma_start(w2o[:], w2o_d[:])
                nc.sync.dma_start(id4[:], id4_d[:])
                nc.sync.dma_start(on16[:], on16_d[:])

            def pool_mms(cx, enm, h1a, ts):
                # enm cols are quarter-major: [Q(4)][i(2)][t_local(16)][h(4)].
                # h1a strips are 65 wide (64 hidden + memset count col), so
                # the normalizers accumulate inside these same matmuls — a
                # separate count accumulation would be wiped by other groups'
                # start flags (PSUM start zeroes its whole 2KB bank).
                for t in ts:
                    for i in (0, 1):
                        ec = 128 * (t // 16) + 64 * i + 4 * (t % 16)
                        k = 8 * (t // 4) + 2 * (t % 4) + i
                        nc.tensor.matmul(
                            cx[0:4, 65 * i : 65 * i + 65],
                            enm[:, ec : ec + 4],
                            h1a[:, 65 * k : 65 * k + 65],
                            start=(t == 0), stop=(t == NB - 1),
                            skip_group_check=True,
                        )

            def epilogue(cx, p):
                rz = smpool.tile([4, 2], f32, name="rz", tag="rz")
                nc.vector.reciprocal(rz[:, 0:1], cx[0:4, 64:65])
                nc.vector.reciprocal(rz[:, 1:2], cx[0:4, 129:130])
                cpn = smpool.tile([4, 128], f32, name="cpn", tag="cpn")
                nc.scalar.activation(cpn[:, 0:64], cx[0:4, 0:64], AF.Copy,
                                     scale=rz[:, 0:1])
                nc.scalar.activation(cpn[:, 64:128], cx[0:4, 65:129], AF.Copy,
                                     scale=rz[:, 1:2])
                tp = cx[:, 130:134]
                nc.tensor.transpose(tp[:], cpn[:], id4[:])
                nc.scalar.activation(
                    ctxnT[0:64, 8 * p : 8 * p + 4], tp[0:64, :], AF.Copy
                )
                nc.scalar.activation(
                    ctxnT[0:64, 8 * p + 4 : 8 * p + 8], tp[64:128, :], AF.Copy
                )

            # Software-pipelined across pairs: in super-iteration q,
            # pair q+1 runs its score matmuls + relus (sh_sb buffered in
            # SBUF for a full pair), pair q runs logits + h1, pair q-1 runs
            # pooling. Removes the relu -> logit latency from the chunk
            # critical path entirely.
            shs = {}   # (pair, chunk) -> relu'd sh_sb tile
            lgs = {}
            enms = {}
            h1as = {}
            cxs = {}

            def sh_step(p, cc):
                # Two 2-bank PSUM tiles per chunk; each relu is issued as
                # soon as its two matmuls land (V takes sAB, S takes sCD), so
                # the relu -> next-chunk-MM cycle (~432+1224+sem) just fits
                # inside the tensor chunk period.
                cs = slice(512 * cc, 512 * (cc + 1))
                xa = xas[p]
                sh_sb = shpool.tile([128, 2048], FP8, name="sh_sb", tag="shs")
                wsp = crit if p == 0 else wsc
                wso = 0 if p == 0 else 512 * p
                sAB = psS2.tile([128, 1024], f32, name="sAB", tag="sAB")
                for s in (0, 1):
                    nc.tensor.matmul(
                        sAB[:, 512 * s : 512 * s + 512],
                        wsp[:, wso + 128 * s : wso + 128 * s + 128],
                        xa[:, cs],
                        start=True, stop=True,
                        skip_group_check=True,
                    )
                nc.scalar.activation(sh_sb[:, 0:1024], sAB[:], AF.Relu)
                for s in (2, 3):
                    sq = psS1.tile([128, 512], f32, name="sq", tag="sq")
                    nc.tensor.matmul(
                        sq[:],
                        wsp[:, wso + 128 * s : wso + 128 * s + 128],
                        xa[:, cs],
                        start=True, stop=True,
                        skip_group_check=True,
                    )
                    nc.vector.tensor_scalar(
                        sh_sb[:, 512 * s : 512 * s + 512], sq[:], 0.0, None,
                        ALU.max,
                    )
                shs[(p, cc)] = sh_sb

            def lg_step(p, cc):
                # Quarter-pair logits: a [128,128] PSUM tile holds 4 chunks'
                # logits, exp'd into its enm slice as soon as it completes.
                sh_sb = shs.pop((p, cc))
                if cc % 4 == 0:
                    lgs[p] = psLG.tile([128, 128], f32, name="lgq", tag="lgq")
                lg = lgs[p]
                for i in (0, 1):
                    for j in range(4):
                        col = 64 * i + 16 * (cc % 4) + 4 * j
                        lgo = lg[:, col : col + 4]
                        base = 1024 * i
                        nc.tensor.matmul(
                            lgo,
                            sh_sb[:, base + 128 * j : base + 128 * j + 128],
                            crit[:, 640:644],
                            start=True, stop=False,
                            skip_group_check=True,
                        )
                        nc.tensor.matmul(
                            lgo,
                            sh_sb[:, base + 512 + 128 * j : base + 512 + 128 * j + 128],
                            crit[:, 644:648],
                            start=False, stop=True,
                            skip_group_check=True,
                        )


            def h1_step(p, cc):
                xa = xas[p]
                h1a = h1as[p]
                h1ab = psH.tile([128, 512], f32, name="h1ab", tag="h1ab")
                wnp = crit if p == 0 else wenm
                wno = 512 if p == 0 else 128 * p
                for j in range(4):
                    t = 4 * cc + j
                    nc.tensor.matmul(
                        h1ab[:, 128 * j : 128 * j + 128],
                        xa[:, 128 * t : 128 * t + 128],
                        wnp[:, wno : wno + 128],
                        start=True, stop=True,
                        skip_group_check=True,
                    )
                dst = h1a.rearrange("p (k c) -> p k c", c=65)[
                    :, 8 * cc : 8 * cc + 8, 0:64
                ]
                # V takes h1 only on flush-exp chunks (1 in 4): a V h1-relu
                # spikes V to ~2.04us (> T period) and with 1-in-2 spacing
                # the backlog never drained (repeating ~2.7us chunks); 3
                # light chunks between spikes let V recover, and S skips h1
                # exactly where its exp lands.
                if cc % 4 == 0:
                    nc.vector.tensor_scalar(dst, h1ab[:], 0.0, None, ALU.max)
                else:
                    nc.scalar.activation(dst, h1ab[:], AF.Relu)

            def open_pair(p):
                enms[p] = enmpool.tile([128, 512], BF16, name="enm", tag="enm")
                h1as[p] = h1pool.tile([128, 128 * 65], BF16, name="h1a", tag="h1a")
                if p < 3:
                    # one-time 1.0 fill of the count columns of each of the
                    # 3 rotating physical buffers; relus never touch them
                    nc.vector.memset(
                        h1as[p].rearrange("p (k c) -> p k c", c=65)[:, :, 64:65],
                        1.0,
                    )

            def flush_exp(p, qq):
                # exp of a completed logit quarter; issued at the TOP of the
                # next chunk so the scalar engine runs it before its relu
                # backlog (frees the single lgq bank for the next quarter)
                nc.scalar.activation(
                    enms[p][:, 128 * qq : 128 * qq + 128], lgs.pop(p)[:],
                    AF.Exp,
                )

            LAG = 6   # pool trails its own pair's logits by 6 chunks
            open_pair(0)
            sh_step(0, 0)
            # The tile scheduler orders by readiness, so to keep these
            # non-critical DMAs from stealing HBM bandwidth during startup,
            # push their scheduler-visible ready time out a few microseconds.
            with tc.tile_wait_until(0.003):
                for k in (1, 2, 3):
                    sl = slice(k * (N // 4), (k + 1) * (N // 4))
                    nc.sync.dma_start(xas[0][:, sl], xpa_d[0][:, sl])
            with tc.tile_wait_until(0.006):
                if PAIRS > 1:
                    load_xa(1)
                late_dmas()
            for cc in range(1, NCHB):
                sh_step(0, cc)
            for q in range(PAIRS):
                # super-iteration q: sh for q+1, logits/h1 for q, pool for q
                # (LAG chunks behind), tail of pool(q-1) + its epilogue in
                # the first LAG chunks.
                if q + 1 < PAIRS:
                    open_pair(q + 1)
                if q + 2 < PAIRS:
                    load_xa(q + 2)
                for cc in range(NCHB):
                    if cc % 4 == 0 and cc > 0:
                        flush_exp(q, cc // 4 - 1)
                    if q + 1 < PAIRS:
                        sh_step(q + 1, cc)
                    lg_step(q, cc)
                    if q >= 1 and cc < LAG:
                        c2 = NCHB - LAG + cc
                        pool_mms(cxs[q - 1], enms[q - 1], h1as[q - 1],
                                 (4 * c2, 4 * c2 + 1, 4 * c2 + 2, 4 * c2 + 3))
                        if cc == LAG - 1:
                            epilogue(cxs.pop(q - 1), q - 1)
                            enms.pop(q - 1)
                            h1as.pop(q - 1)
                    if cc >= LAG:
                        if cc == LAG:
                            cxs[q] = psCX.tile([128, 140], f32, name="cx",
                                               tag="cx")
                        c2 = cc - LAG
                        pool_mms(cxs[q], enms[q], h1as[q],
                                 (4 * c2, 4 * c2 + 1, 4 * c2 + 2, 4 * c2 + 3))
                    h1_step(q, cc)
                flush_exp(q, 3)
                if q - 1 >= 0:
                    xas.pop(q - 1)

            qf = PAIRS - 1
            for cc in range(NCHB - LAG, NCHB):
                pool_mms(cxs[qf], enms[qf], h1as[qf],
                         (4 * cc, 4 * cc + 1, 4 * cc + 2, 4 * cc + 3))
            epilogue(cxs.pop(qf), qf)

        # ---- Phase D: pooled-context encoder layer 2 + output MLP
        with ExitStack() as pctx:
            psD = pctx.enter_context(tc.tile_pool(name="psD", bufs=1, space="PSUM"))
            fct_ps = psD.tile([64, 64], f32, name="fct_ps", tag="fctp")
            nc.tensor.matmul(fct_ps[:], ew2b[:], ctxnT[:], start=True, stop=True)
            nc.vector.tensor_copy(out=fct[:], in_=fct_ps[:])
            fct_bh = fct.rearrange("d (b h) -> d b h", h=HEADS)
            o1_ps = psD.tile([64, BPC], f32, name="o1_ps", tag="o1p")
            for h in range(HEADS):
                nc.tensor.matmul(
                    o1_ps[:],
                    ow1[:, h * 64 : (h + 1) * 64],
                    fct_bh[:, :, h],
                    start=(h == 0),
                    stop=False,
                    skip_group_check=True,
                )
            nc.tensor.matmul(
                o1_ps[:], ob1[:], on16[:], start=False, stop=True,
                skip_group_check=True,
            )
            nc.scalar.activation(obuf[0:64, :], o1_ps[:], AF.Relu)
            fin_ps = psD.tile([1, BPC], f32, name="fin_ps", tag="finp")
            nc.tensor.matmul(fin_ps[:], w2o[:], obuf[:], start=True, stop=True)
            nc.vector.tensor_copy(out=res[:], in_=fin_ps[:])
            nc.sync.dma_start(out_d.rearrange("(a n) -> a n", a=1), res[:])

    if not nc.is_finalized():
        nc.finalize()
    return nc


def make_in_maps(inputs):
    """Host-side marshalling: fp8 hi/lo packing; radar folded into biases;
    point rows replicated across all 128 partitions (8 copies per b)."""
    import ml_dtypes

    f8 = ml_dtypes.float8_e4m3fn
    f = np.float32

    def split8(a):
        hi = a.astype(f8)
        lo = (a - hi.astype(f)).astype(f8)
        return hi, lo

    radar = np.concatenate(
        [np.asarray(inputs["radar_xy"], f), np.asarray(inputs["radar_dir"], f)], axis=1
    )
    pts = np.asarray(inputs["pts"], f)
    enc_w1 = np.asarray(inputs["enc_w1"], f)
    enc_b1 = np.asarray(inputs["enc_b1"], f)
    enc_w2 = np.asarray(inputs["enc_w2"], f)
    enc_b2 = np.asarray(inputs["enc_b2"], f)
    sc_w1 = np.asarray(inputs["sc_w1"], f)
    sc_b1 = np.asarray(inputs["sc_b1"], f)
    sc_w2 = np.asarray(inputs["sc_w2"], f)
    out_w1 = np.asarray(inputs["out_w1"], f)
    out_b1 = np.asarray(inputs["out_b1"], f)
    out_w2 = np.asarray(inputs["out_w2"], f)
    out_b2 = np.asarray(inputs["out_b2"], f)

    cb_sc = np.einsum("br,hrd->bhd", radar, sc_w1[:, :4, :]) + sc_b1  # [B, 4, 64]
    cb_enc = radar @ enc_w1[:4] + enc_b1  # [B, 64]

    x = pts[:, :, 0]
    y = pts[:, :, 1]
    xh, xl = split8(x)
    yh, yl = split8(y)

    # 8 feature rows per b: [xh, yh, xh, yh, xl, yl, 1, 1]
    xrows = np.empty((B, 8, N), f8)
    xrows[:, 0] = xh
    xrows[:, 1] = yh
    xrows[:, 2] = xh
    xrows[:, 3] = yh
    xrows[:, 4] = xl
    xrows[:, 5] = yl
    xrows[:, 6] = 1.0
    xrows[:, 7] = 1.0

    # score stationary rows: [wxh, wyh, wxl, wyl, wxh, wyh, cbh, cbl]
    wx = sc_w1[:, 4, :]
    wy = sc_w1[:, 5, :]
    wxh_, wxl_ = split8(wx)
    wyh_, wyl_ = split8(wy)

    def sc_stat(b, half, row0):
        st = np.zeros((128, 128), f8)
        for hh in range(2):
            h = half * 2 + hh
            s = slice(hh * 64, hh * 64 + 64)
            st[row0 + 0, s] = wxh_[h]
            st[row0 + 1, s] = wyh_[h]
            st[row0 + 2, s] = wxl_[h]
            st[row0 + 3, s] = wyl_[h]
            st[row0 + 4, s] = wxh_[h]
            st[row0 + 5, s] = wyh_[h]
            cbh, cbl = split8(cb_sc[b, h])
            st[row0 + 6, s] = cbh
            st[row0 + 7, s] = cbl
        return st

    exh_, exl_ = split8(enc_w1[4])
    eyh_, eyl_ = split8(enc_w1[5])

    def enc_rhs(b, row0):
        st = np.zeros((128, 64), f8)
        st[row0 + 0] = exh_
        st[row0 + 1] = eyh_
        st[row0 + 2] = exl_
        st[row0 + 3] = eyl_
        st[row0 + 4] = exh_
        st[row0 + 5] = eyh_
        cbh, cbl = split8(cb_enc[b])
        st[row0 + 6] = cbh
        st[row0 + 7] = cbl
        return st

    w24 = np.zeros((128, 8), f8)
    w24[0:64, 0] = sc_w2[0].astype(f8)
    w24[64:128, 1] = sc_w2[1].astype(f8)
    w24[0:64, 6] = sc_w2[2].astype(f8)
    w24[64:128, 7] = sc_w2[3].astype(f8)

    ew2b = np.concatenate([enc_w2, enc_b2[None, :]], axis=0)
    ow1 = np.empty((64, 256), f)
    for h in range(HEADS):
        ow1[:, h * 64 : (h + 1) * 64] = out_w1[h * 64 : (h + 1) * 64, :]
    ob1 = np.ascontiguousarray(out_b1[None, :])
    w2o = np.concatenate([out_w2, out_b2[None, :]], axis=0)
    id4 = np.eye(4, dtype=f)
    on16 = np.ones((1, BPC), f)

    in_maps = []
    for core in range(NCORES):
        b0 = core * BPC
        xpa = np.empty((PAIRS, 128, N), f8)
        wsc = np.empty((128, PAIRS * 512), f8)
        wenm = np.empty((128, PAIRS * 128), f8)
        for p in range(PAIRS):
            be, bo = b0 + 2 * p, b0 + 2 * p + 1
            xpa[p, 0:64] = np.tile(xrows[be], (8, 1))
            xpa[p, 64:128] = np.tile(xrows[bo], (8, 1))
            wsc[:, 512 * p + 0 : 512 * p + 128] = sc_stat(be, 0, 0)
            wsc[:, 512 * p + 128 : 512 * p + 256] = sc_stat(be, 1, 0)
            wsc[:, 512 * p + 256 : 512 * p + 384] = sc_stat(bo, 0, 64)
            wsc[:, 512 * p + 384 : 512 * p + 512] = sc_stat(bo, 1, 64)
            wenm[:, 128 * p : 128 * p + 64] = enc_rhs(be, 0)
            wenm[:, 128 * p + 64 : 128 * p + 128] = enc_rhs(bo, 64)
        critb = np.concatenate([wsc[:, 0:512], wenm[:, 0:128],
                                w24.reshape(128, 8)], axis=1)
        in_maps.append(
            dict(
                xpa=xpa, crit=critb, wsc=wsc, wenm=wenm, ew2b=ew2b, ow1=ow1,
                ob1=ob1, w2o=w2o, id4=id4, on16=on16,
            )
        )
    return in_maps


_CACHE = {}


def _get_runner():
    if "runner" in _CACHE:
        return _CACHE["runner"]

    import jax
    from jax.sharding import Mesh, NamedSharding, PartitionSpec

    from concourse.bass2jax import (
        _bass_exec_p,
        install_neuronx_cc_hook,
        partition_id_tensor,
        shard_map,
    )

    nc = build_nc()
    _CACHE["nc"] = nc
    install_neuronx_cc_hook()
    partition_name = nc.partition_id_tensor.name if nc.partition_id_tensor else None
    in_names, out_names, out_avals = [], [], []
    for alloc in nc.m.functions[0].allocations:
        if not isinstance(alloc, mybir.MemoryLocationSet):
            continue
        name = alloc.memorylocations[0].name
        if alloc.kind == "ExternalInput":
            if name != partition_name:
                in_names.append(name)
        elif alloc.kind == "ExternalOutput":
            out_names.append(name)
            out_avals.append(
                jax.core.ShapedArray(tuple(alloc.tensor_shape), mybir.dt.np(alloc.dtype))
            )
    all_in_names = tuple(in_names + out_names)
    if partition_name is not None:
        all_in_names = all_in_names + (partition_name,)

    def _body(*args):
        operands = list(args)
        if partition_name is not None:
            operands.append(partition_id_tensor())
        return tuple(
            _bass_exec_p.bind(
                *operands,
                out_avals=tuple(out_avals),
                in_names=all_in_names,
                out_names=tuple(out_names),
                lowering_input_output_aliases=(),
                sim_require_finite=True,
                sim_require_nnan=True,
                nc=nc,
            )
        )

    devices = jax.devices()[:NCORES]
    mesh = Mesh(np.asarray(devices), ("core",))
    nin = len(in_names) + len(out_names)
    fn = jax.jit(
        shard_map(
            _body,
            mesh=mesh,
            in_specs=(PartitionSpec("core"),) * nin,
            out_specs=(PartitionSpec("core"),) * len(out_names),
            check_rep=False,
        ),
        keep_unused=True,
    )
    sharding = NamedSharding(mesh, PartitionSpec("core"))
    runner = (fn, sharding, in_names, out_avals)
    _CACHE["runner"] = runner
    return runner


def kernel(**inputs):
    import jax

    in_maps = make_in_maps(inputs)
    fn, sharding, in_names, out_avals = _get_runner()
    concat_in = [
        np.concatenate([np.asarray(in_maps[c][name]) for c in range(NCORES)], axis=0)
        for name in in_names
    ]
    concat_zeros = [
        np.zeros((NCORES * a.shape[0], *a.shape[1:]), a.dtype) for a in out_avals
    ]
    args = [jax.device_put(a, sharding) for a in (*concat_in, *concat_zeros)]
    (out,) = fn(*args)
    return np.asarray(out).reshape(B).astype(np.float32)

